# revision 33
# baseline (speedup 1.0000x reference)
"""Bahdanau-attention kernel for Trainium2 (8 NeuronCores, data-parallel over batch).

Math (per batch b):
    enc_proj = h_enc @ W1.T + b1          # (L, D)   -- the big matmul
    dec_proj = h_dec @ W2.T + b2          # (D,)
    h        = tanh(enc_proj + dec_proj)  # (L, D)
    scores   = h @ V (+ bv)               # (L,)  bv cancels in softmax, dropped
    attn     = softmax(scores)            # no-max softmax: |scores| small
    ctx      = attn @ enc_proj            # (D,)

Two restructures vs the v1 kernel:

1) identity ctx: since softmax weights sum to 1,
       ctx = attn @ (h_enc @ W1.T + b1) = (attn @ h_enc) @ W1.T + b1
   so the device only produces u = attn @ h_enc (attention-weighted sum of
   RAW encoder states, via fp16) and the tiny (B,D)x(D,D) W1 projection runs
   on the host in fp64.  This decouples the big matmul's precision from ctx:
   enc_proj only feeds the softmax, which tolerates fp8.

2) fp8 DoubleRow big matmul + beta-correction: enc_proj runs in e4m3 pairs
   (2 d-chunks per matmul, 2 MACs/cell/cycle).  The fp8 quantization error
   dx = x8 - x perturbs scores by ~ v . tanh'(x) dx; since E[tanh'] =: beta
   is known, the rank-1 correction  beta * v.(x - x8) = beta[(W1^T v).h16 -
   (W8^T v).h8]  cancels most of it.  Both correction vectors are
   host-computed; the fp8 quantization error of the device c8 vector is
   absorbed into the fp16 a16 vector.  All three score terms (V.tanh partial
   sums, a16 rank-1, c8 rank-1) accumulate into ONE PSUM tile, staged at K=16
   so c8 values sit in e4m3 normal range; exp(scale=1/K .) undoes it.
   Measured in fp64 sim: rel err 0.0097 vs the 2e-2 gate (fp16 gets 0.0010).

Device layout (all transposed, d/e on partitions):
  - h_enc ships HOST-pretransposed twice: fp16 [b, p, k, c, j] for the
    u-chain + a16 rank-1 (sync ring), fp8 same layout for the DoubleRow
    matmuls (gpsimd/SWDGE ring) -- adjacent c-chunk pairs form the DR pair.
  - enc_projT in PSUM via lhsT = W8 pairs [128,2,128], rhs = h8 [128,2,512]
  - tanh on ACT with scale=1/16 (undo the x16 W8 packing) + dec bias
  - scores: V16-weighted partials over e-chunks on DVE, ones-matmul
    contracts partitions, a16/c8 rank-1 matmuls accumulate into the same
    PSUM group; exp(scale=1/16) with accum Z.  Block i-1's scores group is
    emitted right after block i's first c-group.
    (NOTE: scalar_tensor_tensor / tensor_tensor_reduce / memset / SWDGE
    gather-loads hang or crash the HW here -- stick to proven patterns.)
  - u-chain: one fused DVE tensor_tensor over [128, 8, 512] (exp broadcast
    across the chunk dim) + one fused X-reduce into per-block fp32 slots.
  - dec_proj matmul groups interleave into block 0's c-loop (lag 4).
  - LAST block: scores via replicated-V16 matmuls interleaved with the W
    groups, then the a16/c8 terms appended to the same group; u reduction
    splits even/odd chunks between ACT (accum_out) and DVE.
  - divide by Z only at the very end; single end-of-kernel output DMA.
"""

import numpy as np

B, L, D = 32, 2048, 1024
NCORES = 8
NB = B // NCORES  # batches per core
P = 128
NCH = D // P      # 8 chunks of the d/e dimension
NPR = NCH // 2    # 4 DoubleRow pair-chunks
BLK = 512         # l-columns per block (one PSUM bank of fp32)
NBK = L // BLK    # 4 blocks per batch
KSTG = 16.0       # score staging factor
BETA = 0.6        # E[tanh'] correction coefficient
W8SC = 16.0       # W1 fp8 packing scale

_cache = {}


def _build():
    import concourse.bass as bass
    import concourse.tile as tile
    from concourse import bacc, mybir
    from concourse.bass import ts, ds
    from contextlib import ExitStack

    F8 = mybir.dt.float8e4
    FP16 = mybir.dt.float16
    FP32 = mybir.dt.float32
    Alu = mybir.AluOpType
    Act = mybir.ActivationFunctionType
    X = mybir.AxisListType.X
    DR = mybir.MatmulPerfMode.DoubleRow

    nc = bacc.Bacc("TRN2", name="bahdanau_attn")

    henc_t = nc.dram_tensor("henc_t", [NB, P, NBK, NCH, BLK], FP16, kind="ExternalInput")
    henc8 = nc.dram_tensor("henc8", [NB, P, NBK, NCH, BLK], F8, kind="ExternalInput")
    w18 = nc.dram_tensor("w18", [P, NCH, D], F8, kind="ExternalInput")          # [dpart, dchunk, e] = 16*W1T
    # misc: bias(dec_proj+b1+b2, host-computed) | v16 | a16 | c8 | ones
    misc = nc.dram_tensor(
        "misc", [P, NCH * NB + 3 * NCH + P], FP32, kind="ExternalInput"
    )
    out = nc.dram_tensor("u_out", [P, NB, NCH], FP32, kind="ExternalOutput")

    with tile.TileContext(nc) as tc, ExitStack() as ctx:
        wp = ctx.enter_context(tc.tile_pool(name="weights", bufs=1))
        tp = ctx.enter_context(tc.tile_pool(name="hT", bufs=2))
        t8p = ctx.enter_context(tc.tile_pool(name="h8T", bufs=2))
        hp = ctx.enter_context(tc.tile_pool(name="htan", bufs=3))
        sa = ctx.enter_context(tc.tile_pool(name="sacc", bufs=2))
        xp = ctx.enter_context(tc.tile_pool(name="exps", bufs=2))
        up = ctx.enter_context(tc.tile_pool(name="uprod", bufs=2))
        sp = ctx.enter_context(tc.tile_pool(name="scratch", bufs=2))
        fin = ctx.enter_context(tc.tile_pool(name="final", bufs=2))
        psA = ctx.enter_context(tc.tile_pool(name="psA", bufs=5, space="PSUM"))
        psS = ctx.enter_context(tc.tile_pool(name="psS", bufs=2, space="PSUM"))
        psD = ctx.enter_context(tc.tile_pool(name="psD", bufs=1, space="PSUM"))

        # ---- prologue: batched loads on the ACT HWDGE ring ----
        w8_sb = wp.tile([P, NCH, D], F8)
        nc.scalar.dma_start(w8_sb, w18[:])
        misc_sb = wp.tile([P, NCH * NB + 3 * NCH + P], FP32)
        nc.scalar.dma_start(misc_sb, misc[:])
        NBB = NCH * NB
        bias_sb = wp.tile([P, NBB], FP32)
        nc.vector.tensor_copy(bias_sb, misc_sb[:, 0:NBB])
        v_sb = misc_sb[:, NBB : NBB + NCH]          # = 16*V
        a16_sb = misc_sb[:, NBB + NCH : NBB + 2 * NCH]
        c8_sb = misc_sb[:, NBB + 2 * NCH : NBB + 3 * NCH]

        # all-ones lhsT for the cross-partition scores reduction (fp32->fp16)
        ones_sb = wp.tile([P, P], FP16)
        nc.vector.tensor_copy(ones_sb, misc_sb[:, NBB + 3 * NCH :])
        # replicated rank-1 lhsTs: V16 (last block), a16 (fp16), c8 (fp8)
        vrep = wp.tile([P, NCH, P], FP16)
        nc.vector.tensor_copy(vrep, v_sb[:, :, None].to_broadcast([P, NCH, P]))
        arep = wp.tile([P, NCH, P], FP16)
        nc.vector.tensor_copy(arep, a16_sb[:, :, None].to_broadcast([P, NCH, P]))
        crep = wp.tile([P, NCH, P], F8)
        nc.vector.tensor_copy(crep, c8_sb[:, :, None].to_broadcast([P, NCH, P]))


        # ---- software-pipelined main loop over 16 blocks ----
        NBLOCKS = NB * NBK
        LAST = NBLOCKS - 1
        DECLAG = 4
        batch_state = {}
        block_state = {}

        def emit_corrections(i, start, stop):
            """beta-correction rank-1s into block i's score PSUM tile:
            + a16 . h16  and  + c8 . h8 (independent of the tanh path)."""
            bst = block_state[i]
            ps_sc = bst["ps_sc"]
            for c in range(NCH):
                nc.tensor.matmul(
                    ps_sc, lhsT=arep[:, c, :], rhs=bst["hT"][:, c, :],
                    start=(start and c == 0), stop=False,
                )
            for q in range(NPR):
                nc.tensor.matmul(
                    ps_sc, lhsT=crep[:, 2 * q : 2 * q + 2, :],
                    rhs=bst["h8"][:, 2 * q : 2 * q + 2, :],
                    start=False, stop=(stop and q == NPR - 1),
                    perf_mode=DR,
                )

        def front_begin(i):
            """load + first W1 c-group."""
            b, k = divmod(i, NBK)
            if k == 0:
                batch_state[b] = {
                    "exp": xp.tile([P, L], FP16, tag="exp", name="exp_rep"),
                    "zsl": fin.tile([P, NBK], FP32, tag="zsl", name="zsl"),
                    "usl": fin.tile([P, NCH, NBK], FP32, tag="usl", name="u_sl"),
                }
            h8 = t8p.tile([P, NCH, BLK], F8, tag="h8")
            if i == 0:
                # block 0's h8 gates the first matmul: the SWDGE (gpsimd)
                # path has ~12us first-transfer latency, HWDGE ~3us
                nc.sync.dma_start(h8, henc8[b, :, k])
            else:
                nc.gpsimd.dma_start(h8, henc8[b, :, k])
            hT = tp.tile([P, NCH, BLK], FP16, tag="hT")
            nc.sync.dma_start(hT, henc_t[b, :, k])
            block_state[i] = {"hT": hT, "h8": h8, "ps_sc": None, "sacc": None, "htans": []}
            if i == 0:
                # block 0: emit W MM groups ahead of the tanh/sacc parts so
                # dec_proj's groups (gated on the later-arriving w2 pack) can
                # interleave without ever head-of-line blocking the main MMs
                pss = [emit_mms(0, c) for c in range(DECLAG)]
                for c in range(NCH):
                    emit_act(0, c, pss[c])
                    if c + DECLAG < NCH:
                        pss.append(emit_mms(0, c + DECLAG))
            else:
                emit_c_group(i, 0)
            if i == LAST:
                # the rank-1 corrections don't depend on tanh: front-load them
                # so the tail isn't gated on 12 extra matmuls
                block_state[i]["ps_sc"] = psS.tile([P, BLK], FP32, tag="sc", name="ps_sc")
                emit_corrections(i, start=True, stop=False)

        def emit_mms(i, c):
            """one e-chunk's 4 DoubleRow W8 matmuls (pairs of d-chunks)."""
            bst = block_state[i]
            ps = psA.tile([P, BLK], FP32, tag="mm")
            for q in range(NPR):
                nc.tensor.matmul(
                    ps,
                    lhsT=w8_sb[:, 2 * q : 2 * q + 2, ts(c, P)],
                    rhs=bst["h8"][:, 2 * q : 2 * q + 2, :],
                    start=(q == 0),
                    stop=(q == NPR - 1),
                    perf_mode=DR,
                )
            return ps

        def emit_c_group(i, c):
            """one e-chunk: 4 DR matmuls + tanh + scores partial."""
            ps = emit_mms(i, c)
            emit_act(i, c, ps)

        def emit_act(i, c, ps):
            b, k = divmod(i, NBK)
            bst = block_state[i]
            if "htan" not in bst:
                bst["htan"] = hp.tile([P, NCH, BLK], FP16, tag="htan", name="htan_all")
            htan = bst["htan"][:, c, :]
            nc.scalar.activation(
                htan, ps, Act.Tanh, bias=bias_sb[:, b * NCH + c : b * NCH + c + 1], scale=1.0 / W8SC
            )

            if i == LAST:
                # tail path: scores on PE with replicated V16, lagged one group
                # (the correction rank-1s already started this PSUM group)
                if c > 0:
                    nc.tensor.matmul(
                        bst["ps_sc"], lhsT=vrep[:, c - 1, :], rhs=bst["htan"][:, c - 1, :],
                        start=False, stop=False,
                    )
                if c == NCH - 1:
                    nc.tensor.matmul(
                        bst["ps_sc"], lhsT=vrep[:, c, :], rhs=bst["htan"][:, c, :],
                        start=False, stop=True,
                    )
                return
            # V16-weighted partial sums for scores on DVE:
            #   sacc[p, l] = sum_c v16[p, c] * htan_c[p, l]
            with nc.allow_low_precision("fp16 partials; |sacc| < 16"):
                nxt = sa.tile([P, BLK], FP16, tag="sacc")
                if c == 0:
                    nc.vector.tensor_scalar(
                        out=nxt, in0=htan,
                        scalar1=v_sb[:, 0:1], scalar2=None, op0=Alu.mult,
                    )
                else:
                    prod = sa.tile([P, BLK], FP16, tag="sprod")
                    nc.vector.tensor_scalar(
                        out=prod, in0=htan,
                        scalar1=v_sb[:, c : c + 1], scalar2=None, op0=Alu.mult,
                    )
                    nc.vector.tensor_tensor(nxt, prod, bst["sacc"], Alu.add)
                bst["sacc"] = nxt

        def front_rest(i):
            if i == 0:
                return  # block 0 fully emitted in front_begin
            for c in range(1, NCH):
                emit_c_group(i, c)

        def tail_scores(i):
            """scores group of block i: ones-matmul (non-LAST) + a16/c8
            rank-1 corrections, then exp+Z.  Emitted early in block i+1."""
            b, k = divmod(i, NBK)
            st = batch_state[b]
            bst = block_state[i]
            lr = ds(k * BLK, BLK)
            if i != LAST:
                ps_sc = psS.tile([P, BLK], FP32, tag="sc")
                nc.tensor.matmul(ps_sc, lhsT=ones_sb, rhs=bst["sacc"], start=True, stop=False)
                bst["ps_sc"] = ps_sc
                emit_corrections(i, start=False, stop=True)
            ps_sc = bst["ps_sc"]
            nc.scalar.activation(
                st["exp"][:, lr], ps_sc, Act.Exp, scale=1.0 / KSTG,
                accum_out=st["zsl"][:, k : k + 1],
            )

        def tail_u(i):
            """u partials of block i; batch finalize on its last block."""
            b, k = divmod(i, NBK)
            st = batch_state[b]
            bst = block_state[i]
            lr = ds(k * BLK, BLK)
            del block_state[i]
            with nc.allow_low_precision("fp16 block partials; |u_unnorm| < ~1e3"):
                if i == LAST:
                    # tail: per-chunk, reduce on ACT (even) / DVE (odd) so the
                    # two engines pipeline behind the exp
                    for c in range(NCH):
                        scratch = sp.tile([P, BLK], FP16, tag="ttr")
                        nc.vector.tensor_tensor(
                            scratch, bst["hT"][:, c, :], st["exp"][:, lr], Alu.mult
                        )
                        if c % 2 == 0:
                            sink = sp.tile([P, BLK], FP16, tag="ttr2", name="sink")
                            nc.scalar.activation(
                                sink, scratch,
                                Act.Identity, accum_out=st["usl"][:, c, k : k + 1],
                            )
                        else:
                            nc.vector.tensor_reduce(
                                st["usl"][:, c, k : k + 1], scratch, axis=X, op=Alu.add
                            )
                else:
                    prod = up.tile([P, NCH, BLK], FP16, tag="uprod")
                    nc.vector.tensor_tensor(
                        prod, bst["hT"],
                        st["exp"][:, None, lr].to_broadcast([P, NCH, BLK]),
                        Alu.mult,
                    )
                    # split the expensive reduce: chunks 0-3 on DVE (fused),
                    # chunks 4-7 on ACT via identity+accum_out (ACT has slack)
                    nc.vector.tensor_reduce(
                        st["usl"][:, 0:4, k], prod[:, 0:4, :], axis=X, op=Alu.add
                    )
                    for c in range(4, NCH):
                        sink = sp.tile([P, BLK], FP16, tag="ttr2", name="sink")
                        nc.scalar.activation(
                            sink, prod[:, c, :],
                            Act.Identity, accum_out=st["usl"][:, c, k : k + 1],
                        )

            if k == NBK - 1:
                # finalize: u = u_unnorm / Z
                zsum = fin.tile([P, 1], FP32, tag="zsum")
                nc.vector.tensor_reduce(zsum, st["zsl"], axis=X, op=Alu.add)
                recip = fin.tile([P, 1], FP32, tag="recip")
                nc.vector.reciprocal(recip, zsum)
                ured = fin.tile([P, NCH], FP32, tag="ured")
                nc.vector.tensor_reduce(ured, st["usl"], axis=X, op=Alu.add)
                ub = fin.tile([P, NCH], FP32, tag="ub", name="u_batch")
                nc.vector.tensor_scalar(
                    out=ub, in0=ured, scalar1=recip,
                    scalar2=None, op0=Alu.mult,
                )
                nc.sync.dma_start(out[:, b, :], ub)
                del batch_state[b]

        for i in range(NBLOCKS + 1):
            if i < NBLOCKS:
                front_begin(i)
            if i >= 1:
                tail_scores(i - 1)
            if i < NBLOCKS:
                front_rest(i)
            if i >= 1:
                tail_u(i - 1)

    nc.finalize()
    return nc


def _prep_shared(W1, b1, W2, b2, V):
    import ml_dtypes

    f16 = np.float16
    F8 = ml_dtypes.float8_e4m3fn
    # [dpart, dchunk, e] prepacked so the device DMA is contiguous/partition
    w8v_t = (W8SC * W1.T).astype(F8)                       # fp8 values of 16*W1T [d, e]
    w18 = np.ascontiguousarray(
        w8v_t.reshape(NCH, P, D).transpose(1, 0, 2)
    )
    v16t = (KSTG * V).reshape(NCH, P).T.astype(np.float32)
    # beta-correction vectors (d-space)
    W8dq = w8v_t.astype(np.float64).T / W8SC               # [e, d] dequantized
    w1v = (V.astype(np.float64) @ W1.astype(np.float64))   # (D,)
    w8v = (V.astype(np.float64) @ W8dq)
    c8 = (-KSTG * BETA * w8v).astype(F8)                   # device fp8 rank-1 values
    c8dq = c8.astype(np.float64)
    a16 = (KSTG * BETA * w1v - (c8dq + KSTG * BETA * w8v)).astype(f16)
    a16t = a16.astype(np.float32).reshape(NCH, P).T
    c8t = c8dq.astype(np.float32).reshape(NCH, P).T
    return w18, v16t, a16t, c8t


def _prep_misc(core_bias, v16t, a16t, c8t):
    # core_bias: [NB, D] fp64; layout [p, b*NCH + c]: e = c*128 + p
    bt = (
        core_bias.reshape(NB, NCH, P).transpose(2, 0, 1).reshape(P, NB * NCH)
    ).astype(np.float32)
    return np.ascontiguousarray(
        np.concatenate(
            [bt, v16t, a16t, c8t, np.ones((P, P), np.float32)], axis=1
        ).astype(np.float32)
    )


def kernel(h_enc, h_dec, W1, b1, W2, b2, V, bv):
    import ml_dtypes
    from concourse.bass_utils import run_bass_kernel_spmd

    h_enc = np.asarray(h_enc, dtype=np.float32)
    h_dec = np.asarray(h_dec, dtype=np.float32)
    W1 = np.asarray(W1, dtype=np.float32)
    b1 = np.asarray(b1, dtype=np.float32)
    W2 = np.asarray(W2, dtype=np.float32)
    b2 = np.asarray(b2, dtype=np.float32)
    V = np.asarray(V, dtype=np.float32)

    if "nc" not in _cache:
        _cache["nc"] = _build()
    nc = _cache["nc"]

    w18, v16t, a16t, c8t = _prep_shared(W1, b1, W2, b2, V)
    bias_all = (
        h_dec.astype(np.float64) @ W2.astype(np.float64).T
        + b2.astype(np.float64)
        + b1.astype(np.float64)
    )

    # host pre-transpose: henc_t[b, p, k, c, j] = h_enc[b, k*BLK+j, c*128+p]
    hperm = h_enc.reshape(B, NBK, BLK, NCH, P).transpose(0, 4, 1, 3, 2)
    henc_t = np.ascontiguousarray(hperm.astype(np.float16))
    henc8 = np.ascontiguousarray(hperm.astype(ml_dtypes.float8_e4m3fn))

    in_maps = []
    for core in range(NCORES):
        sl = slice(core * NB, (core + 1) * NB)
        in_maps.append(
            {
                "henc_t": henc_t[sl],
                "henc8": henc8[sl],
                "w18": w18,
                "misc": _prep_misc(bias_all[sl], v16t, a16t, c8t),
            }
        )

    res = run_bass_kernel_spmd(nc, in_maps, core_ids=list(range(NCORES)))
    _cache["last_results"] = res
    outs = []
    for core in range(NCORES):
        o = res.results[core]["u_out"]  # [P, NB, NCH]
        outs.append(o.transpose(1, 2, 0).reshape(NB, D))  # d = c*128 + p
    u = np.concatenate(outs, axis=0).astype(np.float64)
    # host finish (fp64): ctx = (attn @ h_enc) @ W1.T + b1
    ctx = u @ W1.astype(np.float64).T + b1.astype(np.float64)
    return ctx.astype(np.float32)


# revision 34
# speedup vs baseline: 1.3145x; 1.3145x over previous
"""Bahdanau-attention kernel for Trainium2 (8 NeuronCores, data-parallel over batch).

Math (per batch b):
    enc_proj = h_enc @ W1.T + b1          # (L, D)   -- the big matmul
    dec_proj = h_dec @ W2.T + b2          # (D,)
    h        = tanh(enc_proj + dec_proj)  # (L, D)
    scores   = h @ V (+ bv)               # (L,)  bv cancels in softmax, dropped
    attn     = softmax(scores)            # no-max softmax: |scores| small
    ctx      = attn @ enc_proj            # (D,)

Two restructures vs the v1 kernel:

1) identity ctx: since softmax weights sum to 1,
       ctx = attn @ (h_enc @ W1.T + b1) = (attn @ h_enc) @ W1.T + b1
   so the device only produces u = attn @ h_enc (attention-weighted sum of
   RAW encoder states, via fp16) and the tiny (B,D)x(D,D) W1 projection runs
   on the host in fp64.  This decouples the big matmul's precision from ctx:
   enc_proj only feeds the softmax, which tolerates fp8.

2) fp8 DoubleRow big matmul + beta-correction: enc_proj runs in e4m3 pairs
   (2 d-chunks per matmul, 2 MACs/cell/cycle).  The fp8 quantization error
   dx = x8 - x perturbs scores by ~ v . tanh'(x) dx; since E[tanh'] =: beta
   is known, the rank-1 correction  beta * v.(x - x8) = beta[(W1^T v).h16 -
   (W8^T v).h8]  cancels most of it.  Both correction vectors are
   host-computed; the fp8 quantization error of the device c8 vector is
   absorbed into the fp16 a16 vector.  All three score terms (V.tanh partial
   sums, a16 rank-1, c8 rank-1) accumulate into ONE PSUM tile, staged at K=16
   so c8 values sit in e4m3 normal range; exp(scale=1/K .) undoes it.
   Measured in fp64 sim: rel err 0.0097 vs the 2e-2 gate (fp16 gets 0.0010).

Device layout (all transposed, d/e on partitions):
  - h_enc ships HOST-pretransposed twice: fp16 [b, p, k, c, j] for the
    u-chain + a16 rank-1 (sync ring), fp8 same layout for the DoubleRow
    matmuls (gpsimd/SWDGE ring) -- adjacent c-chunk pairs form the DR pair.
  - enc_projT in PSUM via lhsT = W8 pairs [128,2,128], rhs = h8 [128,2,512]
  - tanh on ACT with scale=1/16 (undo the x16 W8 packing) + dec bias
  - scores: V16-weighted partials over e-chunks on DVE, ones-matmul
    contracts partitions, a16/c8 rank-1 matmuls accumulate into the same
    PSUM group; exp(scale=1/16) with accum Z.  Block i-1's scores group is
    emitted right after block i's first c-group.
    (NOTE: scalar_tensor_tensor / tensor_tensor_reduce / memset / SWDGE
    gather-loads hang or crash the HW here -- stick to proven patterns.)
  - u-chain: one fused DVE tensor_tensor over [128, 8, 512] (exp broadcast
    across the chunk dim) + one fused X-reduce into per-block fp32 slots.
  - dec_proj matmul groups interleave into block 0's c-loop (lag 4).
  - LAST block: scores via replicated-V16 matmuls interleaved with the W
    groups, then the a16/c8 terms appended to the same group; u reduction
    splits even/odd chunks between ACT (accum_out) and DVE.
  - divide by Z only at the very end; single end-of-kernel output DMA.
"""

import numpy as np

B, L, D = 32, 2048, 1024
NCORES = 8
NB = B // NCORES  # batches per core
P = 128
NCH = D // P      # 8 chunks of the d/e dimension
NPR = NCH // 2    # 4 DoubleRow pair-chunks
BLK = 512         # l-columns per block (one PSUM bank of fp32)
NBK = L // BLK    # 4 blocks per batch
KSTG = 16.0       # score staging factor
BETA = 0.6        # E[tanh'] correction coefficient
W8SC = 16.0       # W1 fp8 packing scale

_cache = {}


def _build():
    import concourse.bass as bass
    import concourse.tile as tile
    from concourse import bacc, mybir
    from concourse.bass import ts, ds
    from contextlib import ExitStack

    F8 = mybir.dt.float8e4
    FP16 = mybir.dt.float16
    FP32 = mybir.dt.float32
    Alu = mybir.AluOpType
    Act = mybir.ActivationFunctionType
    X = mybir.AxisListType.X
    DR = mybir.MatmulPerfMode.DoubleRow

    nc = bacc.Bacc("TRN2", name="bahdanau_attn")

    henc_t = nc.dram_tensor("henc_t", [NB, P, NBK, NCH, BLK], FP16, kind="ExternalInput")
    henc8 = nc.dram_tensor("henc8", [NB, P, NBK, NCH, BLK], F8, kind="ExternalInput")
    w18 = nc.dram_tensor("w18", [P, NCH, D], F8, kind="ExternalInput")          # [dpart, dchunk, e] = 16*W1T
    # misc: bias(dec_proj+b1+b2, host-computed) | v16 | a16 | c8 | ones
    misc = nc.dram_tensor(
        "misc", [P, NCH * NB + 3 * NCH + P], FP32, kind="ExternalInput"
    )
    out = nc.dram_tensor("u_out", [P, NB, NCH], FP32, kind="ExternalOutput")

    with tile.TileContext(nc) as tc, ExitStack() as ctx:
        wp = ctx.enter_context(tc.tile_pool(name="weights", bufs=1))
        tp = ctx.enter_context(tc.tile_pool(name="hT", bufs=3))
        t8p = ctx.enter_context(tc.tile_pool(name="h8T", bufs=3))
        hp = ctx.enter_context(tc.tile_pool(name="htan", bufs=3))
        sa = ctx.enter_context(tc.tile_pool(name="sacc", bufs=2))
        xp = ctx.enter_context(tc.tile_pool(name="exps", bufs=2))
        up = ctx.enter_context(tc.tile_pool(name="uprod", bufs=2))
        sp = ctx.enter_context(tc.tile_pool(name="scratch", bufs=2))
        fin = ctx.enter_context(tc.tile_pool(name="final", bufs=2))
        psA = ctx.enter_context(tc.tile_pool(name="psA", bufs=5, space="PSUM"))
        psS = ctx.enter_context(tc.tile_pool(name="psS", bufs=2, space="PSUM"))
        psD = ctx.enter_context(tc.tile_pool(name="psD", bufs=1, space="PSUM"))

        # ---- prologue: batched loads on the ACT HWDGE ring ----
        w8_sb = wp.tile([P, NCH, D], F8)
        nc.sync.dma_start(w8_sb, w18[:])
        misc_sb = wp.tile([P, NCH * NB + 3 * NCH + P], FP32)
        nc.scalar.dma_start(misc_sb, misc[:])
        NBB = NCH * NB
        bias_sb = wp.tile([P, NBB], FP32)
        nc.vector.tensor_copy(bias_sb, misc_sb[:, 0:NBB])
        v_sb = misc_sb[:, NBB : NBB + NCH]          # = 16*V
        a16_sb = misc_sb[:, NBB + NCH : NBB + 2 * NCH]
        c8_sb = misc_sb[:, NBB + 2 * NCH : NBB + 3 * NCH]

        # all-ones lhsT for the cross-partition scores reduction (fp32->fp16)
        ones_sb = wp.tile([P, P], FP16)
        nc.vector.tensor_copy(ones_sb, misc_sb[:, NBB + 3 * NCH :])
        # replicated rank-1 lhsTs: V16 (last block), a16 (fp16), c8 (fp8)
        vrep = wp.tile([P, NCH, P], FP16)
        nc.vector.tensor_copy(vrep, v_sb[:, :, None].to_broadcast([P, NCH, P]))
        arep = wp.tile([P, NCH, P], FP16)
        nc.vector.tensor_copy(arep, a16_sb[:, :, None].to_broadcast([P, NCH, P]))
        crep = wp.tile([P, NCH, P], F8)
        nc.vector.tensor_copy(crep, c8_sb[:, :, None].to_broadcast([P, NCH, P]))


        # ---- software-pipelined main loop over 16 blocks ----
        NBLOCKS = NB * NBK
        LAST = NBLOCKS - 1
        DECLAG = 4
        batch_state = {}
        block_state = {}

        def emit_corrections(i, start, stop):
            """beta-correction rank-1s into block i's score PSUM tile:
            + a16 . h16  and  + c8 . h8 (independent of the tanh path)."""
            bst = block_state[i]
            ps_sc = bst["ps_sc"]
            for c in range(NCH):
                nc.tensor.matmul(
                    ps_sc, lhsT=arep[:, c, :], rhs=bst["hT"][:, c, :],
                    start=(start and c == 0), stop=False,
                )
            for q in range(NPR):
                nc.tensor.matmul(
                    ps_sc, lhsT=crep[:, 2 * q : 2 * q + 2, :],
                    rhs=bst["h8"][:, 2 * q : 2 * q + 2, :],
                    start=False, stop=(stop and q == NPR - 1),
                    perf_mode=DR,
                )

        def front_begin(i):
            """load + first W1 c-group."""
            b, k = divmod(i, NBK)
            if k == 0:
                batch_state[b] = {
                    "exp": xp.tile([P, L], FP16, tag="exp", name="exp_rep"),
                    "zsl": fin.tile([P, NBK], FP32, tag="zsl", name="zsl"),
                    "usl": fin.tile([P, NCH, NBK], FP32, tag="usl", name="u_sl"),
                }
            h8 = t8p.tile([P, NCH, BLK], F8, tag="h8")
            if i == 0:
                # block 0's h8 gates the first matmul: the SWDGE (gpsimd)
                # path has ~12us first-transfer latency, HWDGE ~3us
                nc.sync.dma_start(h8, henc8[b, :, k])
            else:
                nc.gpsimd.dma_start(h8, henc8[b, :, k])
            hT = tp.tile([P, NCH, BLK], FP16, tag="hT")
            nc.sync.dma_start(hT, henc_t[b, :, k])
            block_state[i] = {"hT": hT, "h8": h8, "ps_sc": None, "sacc": None, "htans": []}
            if i == 0:
                # block 0: emit W MM groups ahead of the tanh/sacc parts so
                # dec_proj's groups (gated on the later-arriving w2 pack) can
                # interleave without ever head-of-line blocking the main MMs
                pss = [emit_mms(0, c) for c in range(DECLAG)]
                for c in range(NCH):
                    emit_act(0, c, pss[c])
                    if c + DECLAG < NCH:
                        pss.append(emit_mms(0, c + DECLAG))
            else:
                emit_c_group(i, 0)
            if i == LAST:
                # the rank-1 corrections don't depend on tanh: front-load them
                # so the tail isn't gated on 12 extra matmuls
                block_state[i]["ps_sc"] = psS.tile([P, BLK], FP32, tag="sc", name="ps_sc")
                emit_corrections(i, start=True, stop=False)

        def emit_mms(i, c):
            """one e-chunk's 4 DoubleRow W8 matmuls (pairs of d-chunks)."""
            bst = block_state[i]
            ps = psA.tile([P, BLK], FP32, tag="mm")
            for q in range(NPR):
                nc.tensor.matmul(
                    ps,
                    lhsT=w8_sb[:, 2 * q : 2 * q + 2, ts(c, P)],
                    rhs=bst["h8"][:, 2 * q : 2 * q + 2, :],
                    start=(q == 0),
                    stop=(q == NPR - 1),
                    perf_mode=DR,
                )
            return ps

        def emit_c_group(i, c):
            """one e-chunk: 4 DR matmuls + tanh + scores partial."""
            ps = emit_mms(i, c)
            emit_act(i, c, ps)

        def emit_act(i, c, ps):
            b, k = divmod(i, NBK)
            bst = block_state[i]
            if "htan" not in bst:
                bst["htan"] = hp.tile([P, NCH, BLK], FP16, tag="htan", name="htan_all")
            htan = bst["htan"][:, c, :]
            nc.scalar.activation(
                htan, ps, Act.Tanh, bias=bias_sb[:, b * NCH + c : b * NCH + c + 1], scale=1.0 / W8SC
            )

            if i == LAST:
                # tail path: scores on PE with replicated V16, lagged one group
                # (the correction rank-1s already started this PSUM group)
                if c > 0:
                    nc.tensor.matmul(
                        bst["ps_sc"], lhsT=vrep[:, c - 1, :], rhs=bst["htan"][:, c - 1, :],
                        start=False, stop=False,
                    )
                if c == NCH - 1:
                    nc.tensor.matmul(
                        bst["ps_sc"], lhsT=vrep[:, c, :], rhs=bst["htan"][:, c, :],
                        start=False, stop=True,
                    )
                return
            # V16-weighted partial sums for scores on DVE:
            #   sacc[p, l] = sum_c v16[p, c] * htan_c[p, l]
            with nc.allow_low_precision("fp16 partials; |sacc| < 16"):
                nxt = sa.tile([P, BLK], FP16, tag="sacc")
                if c == 0:
                    nc.vector.tensor_scalar(
                        out=nxt, in0=htan,
                        scalar1=v_sb[:, 0:1], scalar2=None, op0=Alu.mult,
                    )
                else:
                    prod = sa.tile([P, BLK], FP16, tag="sprod")
                    nc.vector.tensor_scalar(
                        out=prod, in0=htan,
                        scalar1=v_sb[:, c : c + 1], scalar2=None, op0=Alu.mult,
                    )
                    nc.vector.tensor_tensor(nxt, prod, bst["sacc"], Alu.add)
                bst["sacc"] = nxt

        def front_rest(i):
            if i == 0:
                return  # block 0 fully emitted in front_begin
            for c in range(1, NCH):
                emit_c_group(i, c)

        def tail_scores(i):
            """scores group of block i: ones-matmul (non-LAST) + a16/c8
            rank-1 corrections, then exp+Z.  Emitted early in block i+1."""
            b, k = divmod(i, NBK)
            st = batch_state[b]
            bst = block_state[i]
            lr = ds(k * BLK, BLK)
            if i != LAST:
                ps_sc = psS.tile([P, BLK], FP32, tag="sc")
                nc.tensor.matmul(ps_sc, lhsT=ones_sb, rhs=bst["sacc"], start=True, stop=False)
                bst["ps_sc"] = ps_sc
                emit_corrections(i, start=False, stop=True)
            ps_sc = bst["ps_sc"]
            nc.scalar.activation(
                st["exp"][:, lr], ps_sc, Act.Exp, scale=1.0 / KSTG,
                accum_out=st["zsl"][:, k : k + 1],
            )

        def tail_u(i):
            """u partials of block i; batch finalize on its last block."""
            b, k = divmod(i, NBK)
            st = batch_state[b]
            bst = block_state[i]
            lr = ds(k * BLK, BLK)
            del block_state[i]
            with nc.allow_low_precision("fp16 block partials; |u_unnorm| < ~1e3"):
                if i == LAST:
                    # tail: per-chunk, reduce on ACT (even) / DVE (odd) so the
                    # two engines pipeline behind the exp
                    for c in range(NCH):
                        scratch = sp.tile([P, BLK], FP16, tag="ttr")
                        nc.vector.tensor_tensor(
                            scratch, bst["hT"][:, c, :], st["exp"][:, lr], Alu.mult
                        )
                        if c % 2 == 0:
                            sink = sp.tile([P, BLK], FP16, tag="ttr2", name="sink")
                            nc.scalar.activation(
                                sink, scratch,
                                Act.Identity, accum_out=st["usl"][:, c, k : k + 1],
                            )
                        else:
                            nc.vector.tensor_reduce(
                                st["usl"][:, c, k : k + 1], scratch, axis=X, op=Alu.add
                            )
                else:
                    prod = up.tile([P, NCH, BLK], FP16, tag="uprod")
                    nc.vector.tensor_tensor(
                        prod, bst["hT"],
                        st["exp"][:, None, lr].to_broadcast([P, NCH, BLK]),
                        Alu.mult,
                    )
                    # split the expensive reduce: chunks 0-3 on DVE (fused),
                    # chunks 4-7 on ACT via identity+accum_out (ACT has slack)
                    nc.vector.tensor_reduce(
                        st["usl"][:, 0:4, k], prod[:, 0:4, :], axis=X, op=Alu.add
                    )
                    for c in range(4, NCH):
                        sink = sp.tile([P, BLK], FP16, tag="ttr2", name="sink")
                        nc.scalar.activation(
                            sink, prod[:, c, :],
                            Act.Identity, accum_out=st["usl"][:, c, k : k + 1],
                        )

            if k == NBK - 1:
                # finalize: u = u_unnorm / Z
                zsum = fin.tile([P, 1], FP32, tag="zsum")
                nc.vector.tensor_reduce(zsum, st["zsl"], axis=X, op=Alu.add)
                recip = fin.tile([P, 1], FP32, tag="recip")
                nc.vector.reciprocal(recip, zsum)
                ured = fin.tile([P, NCH], FP32, tag="ured")
                nc.vector.tensor_reduce(ured, st["usl"], axis=X, op=Alu.add)
                ub = fin.tile([P, NCH], FP32, tag="ub", name="u_batch")
                nc.vector.tensor_scalar(
                    out=ub, in0=ured, scalar1=recip,
                    scalar2=None, op0=Alu.mult,
                )
                nc.sync.dma_start(out[:, b, :], ub)
                del batch_state[b]

        for i in range(NBLOCKS + 1):
            if i < NBLOCKS:
                front_begin(i)
            if i >= 1:
                tail_scores(i - 1)
            if i < NBLOCKS:
                front_rest(i)
            if i >= 1:
                tail_u(i - 1)

    nc.finalize()
    return nc


def _prep_shared(W1, b1, W2, b2, V):
    import ml_dtypes

    f16 = np.float16
    F8 = ml_dtypes.float8_e4m3fn
    # [dpart, dchunk, e] prepacked so the device DMA is contiguous/partition
    w8v_t = (W8SC * W1.T).astype(F8)                       # fp8 values of 16*W1T [d, e]
    w18 = np.ascontiguousarray(
        w8v_t.reshape(NCH, P, D).transpose(1, 0, 2)
    )
    v16t = (KSTG * V).reshape(NCH, P).T.astype(np.float32)
    # beta-correction vectors (d-space)
    W8dq = w8v_t.astype(np.float64).T / W8SC               # [e, d] dequantized
    w1v = (V.astype(np.float64) @ W1.astype(np.float64))   # (D,)
    w8v = (V.astype(np.float64) @ W8dq)
    c8 = (-KSTG * BETA * w8v).astype(F8)                   # device fp8 rank-1 values
    c8dq = c8.astype(np.float64)
    a16 = (KSTG * BETA * w1v - (c8dq + KSTG * BETA * w8v)).astype(f16)
    a16t = a16.astype(np.float32).reshape(NCH, P).T
    c8t = c8dq.astype(np.float32).reshape(NCH, P).T
    return w18, v16t, a16t, c8t


def _prep_misc(core_bias, v16t, a16t, c8t):
    # core_bias: [NB, D] fp64; layout [p, b*NCH + c]: e = c*128 + p
    bt = (
        core_bias.reshape(NB, NCH, P).transpose(2, 0, 1).reshape(P, NB * NCH)
    ).astype(np.float32)
    return np.ascontiguousarray(
        np.concatenate(
            [bt, v16t, a16t, c8t, np.ones((P, P), np.float32)], axis=1
        ).astype(np.float32)
    )


def kernel(h_enc, h_dec, W1, b1, W2, b2, V, bv):
    import ml_dtypes
    from concourse.bass_utils import run_bass_kernel_spmd

    h_enc = np.asarray(h_enc, dtype=np.float32)
    h_dec = np.asarray(h_dec, dtype=np.float32)
    W1 = np.asarray(W1, dtype=np.float32)
    b1 = np.asarray(b1, dtype=np.float32)
    W2 = np.asarray(W2, dtype=np.float32)
    b2 = np.asarray(b2, dtype=np.float32)
    V = np.asarray(V, dtype=np.float32)

    if "nc" not in _cache:
        _cache["nc"] = _build()
    nc = _cache["nc"]

    w18, v16t, a16t, c8t = _prep_shared(W1, b1, W2, b2, V)
    bias_all = (
        h_dec.astype(np.float64) @ W2.astype(np.float64).T
        + b2.astype(np.float64)
        + b1.astype(np.float64)
    )

    # host pre-transpose: henc_t[b, p, k, c, j] = h_enc[b, k*BLK+j, c*128+p]
    hperm = h_enc.reshape(B, NBK, BLK, NCH, P).transpose(0, 4, 1, 3, 2)
    henc_t = np.ascontiguousarray(hperm.astype(np.float16))
    henc8 = np.ascontiguousarray(hperm.astype(ml_dtypes.float8_e4m3fn))

    in_maps = []
    for core in range(NCORES):
        sl = slice(core * NB, (core + 1) * NB)
        in_maps.append(
            {
                "henc_t": henc_t[sl],
                "henc8": henc8[sl],
                "w18": w18,
                "misc": _prep_misc(bias_all[sl], v16t, a16t, c8t),
            }
        )

    res = run_bass_kernel_spmd(nc, in_maps, core_ids=list(range(NCORES)))
    _cache["last_results"] = res
    outs = []
    for core in range(NCORES):
        o = res.results[core]["u_out"]  # [P, NB, NCH]
        outs.append(o.transpose(1, 2, 0).reshape(NB, D))  # d = c*128 + p
    u = np.concatenate(outs, axis=0).astype(np.float64)
    # host finish (fp64): ctx = (attn @ h_enc) @ W1.T + b1
    ctx = u @ W1.astype(np.float64).T + b1.astype(np.float64)
    return ctx.astype(np.float32)
